# revision 36
# baseline (speedup 1.0000x reference)
"""Multi-head causal attention (B=4, T=2048, C=1024, H=16) on 8 TRN2 NeuronCores.

Sharding: data-parallel over batch (4) x tensor-parallel over heads (2 groups
of 8 heads). Core c handles batch c%4, head-group c//4. Each core:
  - Q/K projections in fp8e4m3 DoubleRow matmuls (2x PE rate, 256-deep
    contraction per pass); weights host-scaled by 8, rescaled in the
    psum->SBUF bias-add copy. V projection in fp16 (accuracy-critical path).
  - Causal flash-style attention per head-pair in 2-key-tile supersteps:
    S^T = K^T.T @ Q^T as 64-row-tiled matmul pairs (2 heads concurrent on the
    PE), exp on ScalarE (fp16 out), input mask applied on diagonal supersteps,
    AV + denominator (ones-matmul) as 64-col-tiled matmul pairs accumulated
    over key-pairs. Supersteps batch 2 key tiles per PE-mode window to halve
    tile-mode switches; AV/den flush lags 2 supersteps so exp latency never
    stalls the PE. No max-subtraction (logits are small; fp32 exp can't
    overflow here).
  - Row-parallel output projection producing a partial [T, C] sum; host adds
    the two head-group partials (out bias added on head-group-0 cores only).
"""

import sys

sys.path.insert(0, "/opt/trn_rl_repo")

import numpy as np
import ml_dtypes

import concourse.bacc as bacc
import concourse.tile as tile
from concourse import mybir
from concourse.bass_utils import run_bass_kernel_spmd
from concourse.masks import make_identity

B, T, C, H, D = 4, 2048, 1024, 16, 64
HPC = 8          # heads per core
PAIRS = HPC // 2
CT = C // 128    # bf16/fp16 contraction tiles
CP = C // 256    # fp8 DoubleRow contraction tile-pairs

F32 = mybir.dt.float32
FP16 = mybir.dt.float16
BF16 = mybir.dt.bfloat16
FP8 = mybir.dt.float8e4
NP16 = np.float16
NPBF = ml_dtypes.bfloat16
NP8 = ml_dtypes.float8_e4m3

W8SCALE = 8.0    # host scale on fp8 Q/K weights (keeps them in normal range)

LAST_RESULT = None  # stashed BassKernelResults for test harnesses


def build():
    nc = bacc.Bacc("TRN2", target_bir_lowering=False)

    xT = nc.dram_tensor("xT", [C, T], FP16, kind="ExternalInput")
    xT8 = nc.dram_tensor("xT8", [CP, 128, 2, T], FP8, kind="ExternalInput")
    wqk8 = nc.dram_tensor("wqk8", [8, 128, CP, 2, 128], FP8, kind="ExternalInput")
    wv = nc.dram_tensor("wv", [C, PAIRS * 128], FP16, kind="ExternalInput")
    bqkv = nc.dram_tensor("bqkv", [128, 12], F32, kind="ExternalInput")
    woT = nc.dram_tensor("woT", [512, C], FP16, kind="ExternalInput")
    bo = nc.dram_tensor("bo", [128, C], F32, kind="ExternalInput")
    mask1 = nc.dram_tensor("mask1", [128, 8, 128], BF16, kind="ExternalInput")
    mask2 = nc.dram_tensor("mask2", [128, 8, 256], BF16, kind="ExternalInput")
    out = nc.dram_tensor("out", [T, C], F32, kind="ExternalOutput")

    with tile.TileContext(nc) as tc:
        with tc.tile_pool(name="persist", bufs=1) as pp, \
             tc.tile_pool(name="stream", bufs=2) as sp, \
             tc.tile_pool(name="pss", bufs=3, space="PSUM") as pss, \
             tc.tile_pool(name="psav", bufs=1, space="PSUM") as psav, \
             tc.tile_pool(name="psden", bufs=1, space="PSUM") as psden:

            # ---------------- input DMAs (critical path first) ----------------
            bqkv_sb = pp.tile([128, 12], F32, tag="bqkv", name="bqkv_sb")
            nc.sync.dma_start(out=bqkv_sb, in_=bqkv[:, :])

            wqk_tiles = {}

            def fetch_wqk(mt):
                wm = pp.tile([128, CP, 2, 128], FP8, tag="wm8", bufs=8, name=f"wm8_{mt}")
                nc.sync.dma_start(out=wm, in_=wqk8[mt, :, :, :, :])
                wqk_tiles[mt] = wm

            fetch_wqk(0)
            fetch_wqk(1)
            # fp8 x tiles: Q/K DoubleRow moving operand, quarters so the first
            # m-tile starts after ~1/16 of the x traffic
            xt8_sb = []
            for cp in range(CP):
                t_ = pp.tile([128, 2, T], FP8, tag="xt8", bufs=CP, name=f"xt8_{cp}")
                nc.sync.dma_start(out=t_[:, :, 0:512], in_=xT8[cp, :, :, 0:512])
                xt8_sb.append(t_)
            for cp in range(CP):
                nc.sync.dma_start(out=xt8_sb[cp][:, :, 512:1024],
                                  in_=xT8[cp, :, :, 512:1024])
            for mt in range(2, 4):
                fetch_wqk(mt)
            for cp in range(CP):
                nc.sync.dma_start(out=xt8_sb[cp][:, :, 1024:2048],
                                  in_=xT8[cp, :, :, 1024:2048])
            wv_tiles = {}
            for pr in range(2):
                wm = pp.tile([128, CT, 128], FP16, tag="wmv", bufs=PAIRS, name=f"wmv{pr}")
                nc.sync.dma_start(
                    out=wm,
                    in_=wv[:, pr * 128:(pr + 1) * 128]
                    .rearrange("(n p) m -> p n m", p=128))
                wv_tiles[pr] = wm
            # fp16 x tiles (V projection moving operand), halves; first half
            # ahead of the late-needed weight tiles and masks
            xt_sb = []
            for ct in range(CT):
                t_ = pp.tile([128, T], FP16, tag="xt", bufs=CT, name=f"xt{ct}")
                nc.sync.dma_start(out=t_[:, 0:1024], in_=xT[ct * 128:(ct + 1) * 128, 0:1024])
                xt_sb.append(t_)
            for mt in range(4, 8):
                fetch_wqk(mt)
            m1_sb = pp.tile([128, 8, 128], BF16, tag="m1", name="m1_sb")
            nc.sync.dma_start(out=m1_sb, in_=mask1[:, :, :])
            m2_sb = pp.tile([128, 8, 256], BF16, tag="m2", name="m2_sb")
            nc.sync.dma_start(out=m2_sb, in_=mask2[:, :, :])
            for pr in range(2, PAIRS):
                wm = pp.tile([128, CT, 128], FP16, tag="wmv", bufs=PAIRS, name=f"wmv{pr}")
                nc.sync.dma_start(
                    out=wm,
                    in_=wv[:, pr * 128:(pr + 1) * 128]
                    .rearrange("(n p) m -> p n m", p=128))
                wv_tiles[pr] = wm
            for ct in range(CT):
                nc.sync.dma_start(out=xt_sb[ct][:, 1024:2048],
                                  in_=xT[ct * 128:(ct + 1) * 128, 1024:2048])


            # ---------------- constants / persistent inputs ----------------
            ones_sb = pp.tile([128, 64], BF16, tag="ones", name="ones")
            nc.vector.memset(ones_sb, 1.0)
            ident = pp.tile([128, 128], BF16, tag="ident", name="ident")
            make_identity(nc, ident)
            bo_sb = pp.tile([128, C], F32, tag="bo", name="bo_sb")
            nc.sync.dma_start(out=bo_sb, in_=bo[:, :])
            wo_sb = []
            for it in range(4):
                w = pp.tile([128, C], FP16, tag="wo", bufs=4, name=f"wo{it}")
                nc.sync.dma_start(out=w, in_=woT[it * 128:(it + 1) * 128, :])
                wo_sb.append(w)

            QT = [pp.tile([128, T], FP16, tag="qt", bufs=4, name=f"qt{p}") for p in range(4)]
            KT = [pp.tile([128, T], FP16, tag="kt", bufs=4, name=f"kt{p}") for p in range(4)]
            V = [pp.tile([128, T], BF16, tag="v", bufs=4, name=f"v{p}") for p in range(4)]
            AT = [pp.tile([128, T], FP16, tag="at", bufs=4, name=f"at{p}") for p in range(4)]

            # ------------- Q/K projection: fp8 DoubleRow m-tiles -------------
            def qk_mtile(pr, j):
                mt = pr * 2 + j
                wm = wqk_tiles.pop(mt)
                dst = QT[pr] if j == 0 else KT[pr]
                # psum = W8SCALE * (W x); q additionally pre-scaled 1/8
                scale = (0.125 if j == 0 else 1.0) / W8SCALE
                bcol = pr * 3 + j
                for tch in range(2):
                    ps = pss.tile([128, 1024], F32, tag="pss", name=f"qk_ps{mt}_{tch}")
                    for cp in range(CP):
                        for hf in range(2):
                            t0 = tch * 1024 + hf * 512
                            nc.tensor.matmul(
                                ps[:, hf * 512:(hf + 1) * 512],
                                wm[:, cp, :, :], xt8_sb[cp][:, :, t0:t0 + 512],
                                start=(cp == 0), stop=(cp == CP - 1),
                                perf_mode=mybir.MatmulPerfMode.DoubleRow)
                    nc.vector.tensor_scalar(
                        dst[:, tch * 1024:(tch + 1) * 1024], ps,
                        scale, bqkv_sb[:, bcol:bcol + 1],
                        mybir.AluOpType.mult, mybir.AluOpType.add)

            # ------------- V projection (fp16) + PE transpose to natural -------------
            def v_mtile(pr):
                wm = wv_tiles.pop(pr)
                bcol = pr * 3 + 2
                vt = pp.tile([128, T], BF16, tag="vt", bufs=2, name=f"vt{pr}")
                for tch in range(2):
                    ps = pss.tile([128, 1024], F32, tag="pss", name=f"v_ps{pr}_{tch}")
                    for ct in range(CT):
                        for hf in range(2):
                            t0 = tch * 1024 + hf * 512
                            nc.tensor.matmul(
                                ps[:, hf * 512:(hf + 1) * 512],
                                wm[:, ct, :], xt_sb[ct][:, t0:t0 + 512],
                                start=(ct == 0), stop=(ct == CT - 1))
                    nc.vector.tensor_scalar_add(
                        vt[:, tch * 1024:(tch + 1) * 1024], ps,
                        bqkv_sb[:, bcol:bcol + 1])
                for g in range(4):
                    pst = psav.tile([128, 512], BF16, tag="av", name=f"vtr{pr}_{g}")
                    for c4 in range(4):
                        k = g * 4 + c4
                        nc.tensor.transpose(
                            pst[:, c4 * 128:(c4 + 1) * 128],
                            vt[:, k * 128:(k + 1) * 128], ident)
                    nc.vector.tensor_copy(V[pr][:, g * 512:(g + 1) * 512], pst)

            # ---------------- out-projection (half-tile granularity) ----------------
            def outproj_half(tt, oc):
                po = pss.tile([128, 1024], F32, tag="pss", name=f"op{tt}_{oc}")
                for it in range(4):
                    nc.tensor.matmul(
                        po[:, 0:512],
                        AT[it][:, tt * 128:(tt + 1) * 128],
                        wo_sb[it][:, oc * 512:(oc + 1) * 512],
                        start=(it == 0), stop=(it == 3))
                o = sp.tile([128, 512], F32, tag="o", bufs=3, name=f"o{tt}_{oc}")
                nc.vector.tensor_add(o, po[:, 0:512], bo_sb[:, oc * 512:(oc + 1) * 512])
                nc.sync.dma_start(
                    out=out[tt * 128:(tt + 1) * 128, oc * 512:(oc + 1) * 512], in_=o)

            def outproj(tt):
                outproj_half(tt, 0)
                outproj_half(tt, 1)

            # ---------------- attention unit: 2-key-tile supersteps ----------------
            def attn_unit(qb, p, fillers=(), fill_every=2):
                fillers = list(fillers)
                ntp = 2 * qb + 2
                q0 = qb * 512
                av = psav.tile([128, 512], F32, tag="av", name=f"av{qb}_{p}")
                den = psden.tile([128, 512], F32, tag="den", name=f"den{qb}_{p}")

                def flush(item):
                    t, es, off, w, vss = item
                    st, fin = (t == 0), (t == ntp - 1)
                    for s in range(2):
                        vs = vss[s]
                        for h in range(2):
                            nc.tensor.matmul(
                                av[h * 64:(h + 1) * 64, off + vs:off + w],
                                V[p][:, (2 * t + s) * 128 + h * 64:
                                     (2 * t + s) * 128 + (h + 1) * 64],
                                es[s][:, h, vs:w],
                                start=(st and s == 0), stop=(fin and s == 1),
                                skip_group_check=True)
                        for h in range(2):
                            nc.tensor.matmul(
                                den[h * 64:(h + 1) * 64, off + vs:off + w],
                                ones_sb, es[s][:, h, vs:w],
                                start=(st and s == 0), stop=(fin and s == 1),
                                skip_group_check=True)

                pend = []
                for t in range(ntp):
                    if len(pend) >= 2:
                        flush(pend.pop(0))
                    off = 256 if t == ntp - 1 else 0
                    w = 512 - off
                    qa = q0 + off
                    diag = t >= 2 * qb
                    # slot-keyed psum tiles: heads side by side in the free
                    # dim, so each tile-position matmul pair (h0||h1) writes one
                    # tile and becomes schedulable atomically
                    sst = [pss.tile([128, 2, 512], F32, tag="pss",
                                    name=f"ss{qb}_{p}_{t}_{sl}") for sl in range(2)]
                    vss = [max(0, (2 * t + sl) * 128 - qa) for sl in range(2)]
                    for sl in range(2):
                        k = 2 * t + sl
                        vs = vss[sl]
                        for h in range(2):
                            nc.tensor.matmul(
                                sst[sl][:, h, vs:w],
                                KT[p][h * 64:(h + 1) * 64, k * 128:(k + 1) * 128],
                                QT[p][h * 64:(h + 1) * 64, qa + vs:qa + w],
                                start=True, stop=True)
                    es = []
                    for sl in range(2):
                        vs = vss[sl]
                        e = sp.tile([128, 2, 512], BF16, tag="e", bufs=12,
                                    name=f"e{qb}_{p}_{t}_{sl}")
                        nc.scalar.activation(
                            e[:, :, vs:w], sst[sl][:, :, vs:w],
                            mybir.ActivationFunctionType.Exp)
                        if diag:
                            for h in range(2):
                                if sl == 0:
                                    nc.vector.tensor_mul(e[:, h, 0:128], e[:, h, 0:128],
                                                         m1_sb[:, t, :])
                                else:
                                    nc.vector.tensor_mul(e[:, h, 128:256], e[:, h, 128:256],
                                                         m2_sb[:, t, 128:256])
                        es.append(e)
                    pend.append((t, es, off, w, vss))
                    if len(fillers) > 1 and (t % fill_every == fill_every - 1):
                        fillers.pop(0)()
                if fillers:
                    fillers.pop(0)()
                if fillers:
                    fillers.pop(0)()
                for item in pend:
                    flush(item)
                for f in fillers:
                    f()

                avs = sp.tile([128, 512], F32, tag="avs", bufs=4, name=f"avs{qb}_{p}")
                nc.vector.tensor_copy(avs, av)
                rec = sp.tile([128, 512], F32, tag="rec", bufs=6, name=f"rec{qb}_{p}")
                nc.vector.reciprocal_approx_fast(rec, den)
                nc.vector.tensor_mul(AT[p][:, q0:q0 + 512], avs, rec)

            # ---------------- emission schedule ----------------
            # Projection-heavy prefix (PE-bound overall), then units with op
            # halves as pre-drain fillers to cover the qb-boundary exp waits.
            def oph(tt, oc):
                return lambda: outproj_half(tt, oc)

            qk_mtile(0, 0)
            qk_mtile(0, 1)
            qk_mtile(1, 0)
            qk_mtile(1, 1)
            v_mtile(0)
            qk_mtile(2, 0)
            qk_mtile(2, 1)
            v_mtile(1)
            qk_mtile(3, 0)
            qk_mtile(3, 1)
            v_mtile(2)
            attn_unit(0, 0)
            v_mtile(3)
            attn_unit(0, 1)
            attn_unit(0, 2)
            attn_unit(0, 3)
            attn_unit(1, 0, fillers=[oph(0, 0), oph(0, 1)])
            attn_unit(1, 1, fillers=[oph(1, 0), oph(1, 1)])
            attn_unit(1, 2, fillers=[oph(2, 0), oph(2, 1)])
            attn_unit(1, 3, fillers=[oph(3, 0), oph(3, 1)])
            attn_unit(2, 0, fillers=[oph(4, 0), oph(4, 1), oph(5, 0)])
            attn_unit(2, 1, fillers=[oph(5, 1), oph(6, 0), oph(6, 1)])
            attn_unit(2, 2, fillers=[oph(7, 0)])
            attn_unit(2, 3, fillers=[oph(7, 1)])
            attn_unit(3, 0, fillers=[oph(8, 0), oph(8, 1), oph(9, 0)])
            attn_unit(3, 1, fillers=[oph(9, 1), oph(10, 0), oph(10, 1)])
            attn_unit(3, 2, fillers=[oph(11, 0), oph(11, 1)])
            attn_unit(3, 3)
            for tt in (12, 13, 14, 15):
                outproj(tt)
    nc.finalize()
    return nc


_NC = None


def kernel(x, qkv_w, qkv_b, out_w, out_b, attn_mask):
    global _NC, LAST_RESULT
    if _NC is None:
        _NC = build()

    x = np.asarray(x, dtype=np.float32)
    qkv_w = np.asarray(qkv_w, dtype=np.float32)
    qkv_b = np.asarray(qkv_b, dtype=np.float32)
    out_w = np.asarray(out_w, dtype=np.float32)
    out_b = np.asarray(out_b, dtype=np.float32)
    mask = np.asarray(attn_mask).reshape(T, T).astype(np.float32)

    # diagonal-band mask tiles in e layout [key p, window col c]
    m1 = np.empty((128, 8, 128), dtype=NPBF)
    m2 = np.empty((128, 8, 256), dtype=NPBF)
    for u in range(8):
        K0 = 256 * u
        K1 = K0 + 128
        m1[:, u, :] = mask[K0:K0 + 128, K0:K0 + 128].T.astype(NPBF)
        m2[:, u, :] = mask[K0:K0 + 256, K1:K1 + 128].T.astype(NPBF)

    in_maps = []
    for c in range(8):
        b, hg = c % 4, c // 4
        h0 = hg * HPC

        xt = np.ascontiguousarray(x[b].T).astype(NP16)
        # fp8 interleave: c-index = 256*cp + 128*i + p
        x8 = np.ascontiguousarray(
            x[b].T.reshape(CP, 2, 128, T).transpose(0, 2, 1, 3)).astype(NP8)

        wqk = np.empty((8, 128, CP, 2, 128), dtype=NP8)
        bias_cols = np.empty((128, 12), dtype=np.float32)
        for pr in range(PAIRS):
            r0 = (h0 + 2 * pr) * D
            qrows = qkv_w[r0:r0 + 128] * W8SCALE          # [128m, C]
            krows = qkv_w[C + r0:C + r0 + 128] * W8SCALE
            for j, rows in ((0, qrows), (1, krows)):
                mt = pr * 2 + j
                # rows.T: [C, 128m] -> [CP, 2i, 128p, 128m] -> [128p, CP, 2i, 128m]
                wqk[mt] = rows.T.reshape(CP, 2, 128, 128).transpose(
                    2, 0, 1, 3).astype(NP8)
            bias_cols[:, 3 * pr + 0] = qkv_b[r0:r0 + 128] * 0.125
            bias_cols[:, 3 * pr + 1] = qkv_b[C + r0:C + r0 + 128]
            bias_cols[:, 3 * pr + 2] = qkv_b[2 * C + r0:2 * C + r0 + 128]

        wv_blocks = [qkv_w[2 * C + (h0 + 2 * pr) * D: 2 * C + (h0 + 2 * pr) * D + 128]
                     for pr in range(PAIRS)]
        wv_host = np.ascontiguousarray(
            np.concatenate(wv_blocks, axis=0).T).astype(NP16)

        woT_host = np.ascontiguousarray(
            out_w[:, h0 * D:(h0 + HPC) * D].T).astype(NP16)
        bo_host = (np.tile(out_b, (128, 1)) if hg == 0
                   else np.zeros((128, C), np.float32)).astype(np.float32)

        in_maps.append({
            "xT": xt,
            "xT8": x8,
            "wqk8": np.ascontiguousarray(wqk),
            "wv": wv_host,
            "bqkv": bias_cols,
            "woT": woT_host,
            "bo": bo_host,
            "mask1": m1,
            "mask2": m2,
        })

    LAST_RESULT = run_bass_kernel_spmd(_NC, in_maps, core_ids=list(range(8)))
    res = LAST_RESULT.results
    out = np.empty((B, T, C), dtype=np.float32)
    for b in range(B):
        out[b] = res[b]["out"] + res[b + 4]["out"]
    return out


# revision 37
# speedup vs baseline: 1.0007x; 1.0007x over previous
"""Multi-head causal attention (B=4, T=2048, C=1024, H=16) on 8 TRN2 NeuronCores.

Sharding: data-parallel over batch (4) x tensor-parallel over heads (2 groups
of 8 heads). Core c handles batch c%4, head-group c//4. Each core:
  - Q/K projections in fp8e4m3 DoubleRow matmuls (2x PE rate, 256-deep
    contraction per pass); weights host-scaled by 8, rescaled in the
    psum->SBUF bias-add copy. V projection in fp16 (accuracy-critical path).
  - Causal flash-style attention per head-pair in 2-key-tile supersteps:
    S^T = K^T.T @ Q^T as 64-row-tiled matmul pairs (2 heads concurrent on the
    PE), exp on ScalarE (fp16 out), input mask applied on diagonal supersteps,
    AV + denominator (ones-matmul) as 64-col-tiled matmul pairs accumulated
    over key-pairs. Supersteps batch 2 key tiles per PE-mode window to halve
    tile-mode switches; AV/den flush lags 2 supersteps so exp latency never
    stalls the PE. No max-subtraction (logits are small; fp32 exp can't
    overflow here).
  - Row-parallel output projection producing a partial [T, C] sum; host adds
    the two head-group partials (out bias added on head-group-0 cores only).
"""

import sys

sys.path.insert(0, "/opt/trn_rl_repo")

import numpy as np
import ml_dtypes

import concourse.bacc as bacc
import concourse.tile as tile
from concourse import mybir
from concourse.bass_utils import run_bass_kernel_spmd
from concourse.masks import make_identity

B, T, C, H, D = 4, 2048, 1024, 16, 64
HPC = 8          # heads per core
PAIRS = HPC // 2
CT = C // 128    # bf16/fp16 contraction tiles
CP = C // 256    # fp8 DoubleRow contraction tile-pairs

F32 = mybir.dt.float32
FP16 = mybir.dt.float16
BF16 = mybir.dt.bfloat16
FP8 = mybir.dt.float8e4
NP16 = np.float16
NPBF = ml_dtypes.bfloat16
NP8 = ml_dtypes.float8_e4m3

W8SCALE = 8.0    # host scale on fp8 Q/K weights (keeps them in normal range)

LAST_RESULT = None  # stashed BassKernelResults for test harnesses


def build():
    nc = bacc.Bacc("TRN2", target_bir_lowering=False)

    xT = nc.dram_tensor("xT", [C, T], FP16, kind="ExternalInput")
    xT8 = nc.dram_tensor("xT8", [CP, 128, 2, T], FP8, kind="ExternalInput")
    wqk8 = nc.dram_tensor("wqk8", [8, 128, CP, 2, 128], FP8, kind="ExternalInput")
    wv = nc.dram_tensor("wv", [C, PAIRS * 128], FP16, kind="ExternalInput")
    bqkv = nc.dram_tensor("bqkv", [128, 12], F32, kind="ExternalInput")
    woT = nc.dram_tensor("woT", [512, C], FP16, kind="ExternalInput")
    bo = nc.dram_tensor("bo", [128, C], F32, kind="ExternalInput")
    mask1 = nc.dram_tensor("mask1", [128, 8, 128], BF16, kind="ExternalInput")
    mask2 = nc.dram_tensor("mask2", [128, 8, 256], BF16, kind="ExternalInput")
    out = nc.dram_tensor("out", [T, C], F32, kind="ExternalOutput")

    with tile.TileContext(nc) as tc:
        with tc.tile_pool(name="persist", bufs=1) as pp, \
             tc.tile_pool(name="stream", bufs=2) as sp, \
             tc.tile_pool(name="pss", bufs=3, space="PSUM") as pss, \
             tc.tile_pool(name="psav", bufs=1, space="PSUM") as psav, \
             tc.tile_pool(name="psden", bufs=1, space="PSUM") as psden:

            # ---------------- input DMAs (critical path first) ----------------
            bqkv_sb = pp.tile([128, 12], F32, tag="bqkv", name="bqkv_sb")
            nc.sync.dma_start(out=bqkv_sb, in_=bqkv[:, :])

            wqk_tiles = {}

            def fetch_wqk(mt):
                wm = pp.tile([128, CP, 2, 128], FP8, tag="wm8", bufs=8, name=f"wm8_{mt}")
                nc.sync.dma_start(out=wm, in_=wqk8[mt, :, :, :, :])
                wqk_tiles[mt] = wm

            # fp8 x tiles: Q/K DoubleRow moving operand, quarters so the first
            # m-tile starts after ~1/16 of the x traffic
            xt8_sb = []
            for cp in range(CP):
                t_ = pp.tile([128, 2, T], FP8, tag="xt8", bufs=CP, name=f"xt8_{cp}")
                nc.sync.dma_start(out=t_[:, :, 0:512], in_=xT8[cp, :, :, 0:512])
                xt8_sb.append(t_)
            for cp in range(CP):
                nc.sync.dma_start(out=xt8_sb[cp][:, :, 512:1024],
                                  in_=xT8[cp, :, :, 512:1024])
            for mt in range(4):
                fetch_wqk(mt)
            for cp in range(CP):
                nc.sync.dma_start(out=xt8_sb[cp][:, :, 1024:2048],
                                  in_=xT8[cp, :, :, 1024:2048])
            wv_tiles = {}
            for pr in range(2):
                wm = pp.tile([128, CT, 128], FP16, tag="wmv", bufs=PAIRS, name=f"wmv{pr}")
                nc.sync.dma_start(
                    out=wm,
                    in_=wv[:, pr * 128:(pr + 1) * 128]
                    .rearrange("(n p) m -> p n m", p=128))
                wv_tiles[pr] = wm
            # fp16 x tiles (V projection moving operand), halves; first half
            # ahead of the late-needed weight tiles and masks
            xt_sb = []
            for ct in range(CT):
                t_ = pp.tile([128, T], FP16, tag="xt", bufs=CT, name=f"xt{ct}")
                nc.sync.dma_start(out=t_[:, 0:1024], in_=xT[ct * 128:(ct + 1) * 128, 0:1024])
                xt_sb.append(t_)
            for mt in range(4, 8):
                fetch_wqk(mt)
            m1_sb = pp.tile([128, 8, 128], BF16, tag="m1", name="m1_sb")
            nc.sync.dma_start(out=m1_sb, in_=mask1[:, :, :])
            m2_sb = pp.tile([128, 8, 256], BF16, tag="m2", name="m2_sb")
            nc.sync.dma_start(out=m2_sb, in_=mask2[:, :, :])
            for pr in range(2, PAIRS):
                wm = pp.tile([128, CT, 128], FP16, tag="wmv", bufs=PAIRS, name=f"wmv{pr}")
                nc.sync.dma_start(
                    out=wm,
                    in_=wv[:, pr * 128:(pr + 1) * 128]
                    .rearrange("(n p) m -> p n m", p=128))
                wv_tiles[pr] = wm
            for ct in range(CT):
                nc.sync.dma_start(out=xt_sb[ct][:, 1024:2048],
                                  in_=xT[ct * 128:(ct + 1) * 128, 1024:2048])


            # ---------------- constants / persistent inputs ----------------
            ones_sb = pp.tile([128, 64], BF16, tag="ones", name="ones")
            nc.vector.memset(ones_sb, 1.0)
            ident = pp.tile([128, 128], BF16, tag="ident", name="ident")
            make_identity(nc, ident)
            bo_sb = pp.tile([128, C], F32, tag="bo", name="bo_sb")
            nc.sync.dma_start(out=bo_sb, in_=bo[:, :])
            wo_sb = []
            for it in range(4):
                w = pp.tile([128, C], FP16, tag="wo", bufs=4, name=f"wo{it}")
                nc.sync.dma_start(out=w, in_=woT[it * 128:(it + 1) * 128, :])
                wo_sb.append(w)

            QT = [pp.tile([128, T], FP16, tag="qt", bufs=4, name=f"qt{p}") for p in range(4)]
            KT = [pp.tile([128, T], FP16, tag="kt", bufs=4, name=f"kt{p}") for p in range(4)]
            V = [pp.tile([128, T], BF16, tag="v", bufs=4, name=f"v{p}") for p in range(4)]
            AT = [pp.tile([128, T], FP16, tag="at", bufs=4, name=f"at{p}") for p in range(4)]

            # ------------- Q/K projection: fp8 DoubleRow m-tiles -------------
            def qk_mtile(pr, j):
                mt = pr * 2 + j
                wm = wqk_tiles.pop(mt)
                dst = QT[pr] if j == 0 else KT[pr]
                # psum = W8SCALE * (W x); q additionally pre-scaled 1/8
                scale = (0.125 if j == 0 else 1.0) / W8SCALE
                bcol = pr * 3 + j
                for tch in range(2):
                    ps = pss.tile([128, 1024], F32, tag="pss", name=f"qk_ps{mt}_{tch}")
                    for cp in range(CP):
                        for hf in range(2):
                            t0 = tch * 1024 + hf * 512
                            nc.tensor.matmul(
                                ps[:, hf * 512:(hf + 1) * 512],
                                wm[:, cp, :, :], xt8_sb[cp][:, :, t0:t0 + 512],
                                start=(cp == 0), stop=(cp == CP - 1),
                                perf_mode=mybir.MatmulPerfMode.DoubleRow)
                    nc.vector.tensor_scalar(
                        dst[:, tch * 1024:(tch + 1) * 1024], ps,
                        scale, bqkv_sb[:, bcol:bcol + 1],
                        mybir.AluOpType.mult, mybir.AluOpType.add)

            # ------------- V projection (fp16) + PE transpose to natural -------------
            def v_mtile(pr):
                wm = wv_tiles.pop(pr)
                bcol = pr * 3 + 2
                vt = pp.tile([128, T], BF16, tag="vt", bufs=2, name=f"vt{pr}")
                for tch in range(2):
                    ps = pss.tile([128, 1024], F32, tag="pss", name=f"v_ps{pr}_{tch}")
                    for ct in range(CT):
                        for hf in range(2):
                            t0 = tch * 1024 + hf * 512
                            nc.tensor.matmul(
                                ps[:, hf * 512:(hf + 1) * 512],
                                wm[:, ct, :], xt_sb[ct][:, t0:t0 + 512],
                                start=(ct == 0), stop=(ct == CT - 1))
                    nc.vector.tensor_scalar_add(
                        vt[:, tch * 1024:(tch + 1) * 1024], ps,
                        bqkv_sb[:, bcol:bcol + 1])
                for g in range(4):
                    pst = psav.tile([128, 512], BF16, tag="av", name=f"vtr{pr}_{g}")
                    for c4 in range(4):
                        k = g * 4 + c4
                        nc.tensor.transpose(
                            pst[:, c4 * 128:(c4 + 1) * 128],
                            vt[:, k * 128:(k + 1) * 128], ident)
                    nc.vector.tensor_copy(V[pr][:, g * 512:(g + 1) * 512], pst)

            # ---------------- out-projection (half-tile granularity) ----------------
            def outproj_half(tt, oc):
                po = pss.tile([128, 1024], F32, tag="pss", name=f"op{tt}_{oc}")
                for it in range(4):
                    nc.tensor.matmul(
                        po[:, 0:512],
                        AT[it][:, tt * 128:(tt + 1) * 128],
                        wo_sb[it][:, oc * 512:(oc + 1) * 512],
                        start=(it == 0), stop=(it == 3))
                o = sp.tile([128, 512], F32, tag="o", bufs=3, name=f"o{tt}_{oc}")
                nc.vector.tensor_add(o, po[:, 0:512], bo_sb[:, oc * 512:(oc + 1) * 512])
                nc.sync.dma_start(
                    out=out[tt * 128:(tt + 1) * 128, oc * 512:(oc + 1) * 512], in_=o)

            def outproj(tt):
                outproj_half(tt, 0)
                outproj_half(tt, 1)

            # ---------------- attention unit: 2-key-tile supersteps ----------------
            def attn_unit(qb, p, fillers=(), fill_every=2):
                fillers = list(fillers)
                ntp = 2 * qb + 2
                q0 = qb * 512
                av = psav.tile([128, 512], F32, tag="av", name=f"av{qb}_{p}")
                den = psden.tile([128, 512], F32, tag="den", name=f"den{qb}_{p}")

                def flush(item):
                    t, es, off, w, vss = item
                    st, fin = (t == 0), (t == ntp - 1)
                    for s in range(2):
                        vs = vss[s]
                        for h in range(2):
                            nc.tensor.matmul(
                                av[h * 64:(h + 1) * 64, off + vs:off + w],
                                V[p][:, (2 * t + s) * 128 + h * 64:
                                     (2 * t + s) * 128 + (h + 1) * 64],
                                es[s][:, h, vs:w],
                                start=(st and s == 0), stop=(fin and s == 1),
                                skip_group_check=True)
                        for h in range(2):
                            nc.tensor.matmul(
                                den[h * 64:(h + 1) * 64, off + vs:off + w],
                                ones_sb, es[s][:, h, vs:w],
                                start=(st and s == 0), stop=(fin and s == 1),
                                skip_group_check=True)

                pend = []
                for t in range(ntp):
                    if len(pend) >= 2:
                        flush(pend.pop(0))
                    off = 256 if t == ntp - 1 else 0
                    w = 512 - off
                    qa = q0 + off
                    diag = t >= 2 * qb
                    # slot-keyed psum tiles: heads side by side in the free
                    # dim, so each tile-position matmul pair (h0||h1) writes one
                    # tile and becomes schedulable atomically
                    sst = [pss.tile([128, 2, 512], F32, tag="pss",
                                    name=f"ss{qb}_{p}_{t}_{sl}") for sl in range(2)]
                    vss = [max(0, (2 * t + sl) * 128 - qa) for sl in range(2)]
                    for sl in range(2):
                        k = 2 * t + sl
                        vs = vss[sl]
                        for h in range(2):
                            nc.tensor.matmul(
                                sst[sl][:, h, vs:w],
                                KT[p][h * 64:(h + 1) * 64, k * 128:(k + 1) * 128],
                                QT[p][h * 64:(h + 1) * 64, qa + vs:qa + w],
                                start=True, stop=True)
                    es = []
                    for sl in range(2):
                        vs = vss[sl]
                        e = sp.tile([128, 2, 512], BF16, tag="e", bufs=12,
                                    name=f"e{qb}_{p}_{t}_{sl}")
                        nc.scalar.activation(
                            e[:, :, vs:w], sst[sl][:, :, vs:w],
                            mybir.ActivationFunctionType.Exp)
                        if diag:
                            for h in range(2):
                                if sl == 0:
                                    nc.vector.tensor_mul(e[:, h, 0:128], e[:, h, 0:128],
                                                         m1_sb[:, t, :])
                                else:
                                    nc.vector.tensor_mul(e[:, h, 128:256], e[:, h, 128:256],
                                                         m2_sb[:, t, 128:256])
                        es.append(e)
                    pend.append((t, es, off, w, vss))
                    if len(fillers) > 1 and (t % fill_every == fill_every - 1):
                        fillers.pop(0)()
                if fillers:
                    fillers.pop(0)()
                if fillers:
                    fillers.pop(0)()
                for item in pend:
                    flush(item)
                for f in fillers:
                    f()

                avs = sp.tile([128, 512], F32, tag="avs", bufs=4, name=f"avs{qb}_{p}")
                nc.vector.tensor_copy(avs, av)
                rec = sp.tile([128, 512], F32, tag="rec", bufs=6, name=f"rec{qb}_{p}")
                nc.vector.reciprocal_approx_fast(rec, den)
                nc.vector.tensor_mul(AT[p][:, q0:q0 + 512], avs, rec)

            # ---------------- emission schedule ----------------
            # Projection-heavy prefix (PE-bound overall), then units with op
            # halves as pre-drain fillers to cover the qb-boundary exp waits.
            def oph(tt, oc):
                return lambda: outproj_half(tt, oc)

            qk_mtile(0, 0)
            qk_mtile(0, 1)
            qk_mtile(1, 0)
            qk_mtile(1, 1)
            v_mtile(0)
            qk_mtile(2, 0)
            qk_mtile(2, 1)
            v_mtile(1)
            qk_mtile(3, 0)
            qk_mtile(3, 1)
            v_mtile(2)
            attn_unit(0, 0)
            v_mtile(3)
            attn_unit(0, 1)
            attn_unit(0, 2)
            attn_unit(0, 3)
            attn_unit(1, 0, fillers=[oph(0, 0), oph(0, 1)])
            attn_unit(1, 1, fillers=[oph(1, 0), oph(1, 1)])
            attn_unit(1, 2, fillers=[oph(2, 0), oph(2, 1)])
            attn_unit(1, 3, fillers=[oph(3, 0), oph(3, 1)])
            attn_unit(2, 0, fillers=[oph(4, 0), oph(4, 1), oph(5, 0)])
            attn_unit(2, 1, fillers=[oph(5, 1), oph(6, 0), oph(6, 1)])
            attn_unit(2, 2, fillers=[oph(7, 0)])
            attn_unit(2, 3, fillers=[oph(7, 1)])
            attn_unit(3, 0, fillers=[oph(8, 0), oph(8, 1), oph(9, 0)])
            attn_unit(3, 1, fillers=[oph(9, 1), oph(10, 0), oph(10, 1)])
            attn_unit(3, 2, fillers=[oph(11, 0), oph(11, 1)])
            attn_unit(3, 3)
            for tt in (12, 13, 14, 15):
                outproj(tt)
    nc.finalize()
    return nc


_NC = None


def kernel(x, qkv_w, qkv_b, out_w, out_b, attn_mask):
    global _NC, LAST_RESULT
    if _NC is None:
        _NC = build()

    x = np.asarray(x, dtype=np.float32)
    qkv_w = np.asarray(qkv_w, dtype=np.float32)
    qkv_b = np.asarray(qkv_b, dtype=np.float32)
    out_w = np.asarray(out_w, dtype=np.float32)
    out_b = np.asarray(out_b, dtype=np.float32)
    mask = np.asarray(attn_mask).reshape(T, T).astype(np.float32)

    # diagonal-band mask tiles in e layout [key p, window col c]
    m1 = np.empty((128, 8, 128), dtype=NPBF)
    m2 = np.empty((128, 8, 256), dtype=NPBF)
    for u in range(8):
        K0 = 256 * u
        K1 = K0 + 128
        m1[:, u, :] = mask[K0:K0 + 128, K0:K0 + 128].T.astype(NPBF)
        m2[:, u, :] = mask[K0:K0 + 256, K1:K1 + 128].T.astype(NPBF)

    in_maps = []
    for c in range(8):
        b, hg = c % 4, c // 4
        h0 = hg * HPC

        xt = np.ascontiguousarray(x[b].T).astype(NP16)
        # fp8 interleave: c-index = 256*cp + 128*i + p
        x8 = np.ascontiguousarray(
            x[b].T.reshape(CP, 2, 128, T).transpose(0, 2, 1, 3)).astype(NP8)

        wqk = np.empty((8, 128, CP, 2, 128), dtype=NP8)
        bias_cols = np.empty((128, 12), dtype=np.float32)
        for pr in range(PAIRS):
            r0 = (h0 + 2 * pr) * D
            qrows = qkv_w[r0:r0 + 128] * W8SCALE          # [128m, C]
            krows = qkv_w[C + r0:C + r0 + 128] * W8SCALE
            for j, rows in ((0, qrows), (1, krows)):
                mt = pr * 2 + j
                # rows.T: [C, 128m] -> [CP, 2i, 128p, 128m] -> [128p, CP, 2i, 128m]
                wqk[mt] = rows.T.reshape(CP, 2, 128, 128).transpose(
                    2, 0, 1, 3).astype(NP8)
            bias_cols[:, 3 * pr + 0] = qkv_b[r0:r0 + 128] * 0.125
            bias_cols[:, 3 * pr + 1] = qkv_b[C + r0:C + r0 + 128]
            bias_cols[:, 3 * pr + 2] = qkv_b[2 * C + r0:2 * C + r0 + 128]

        wv_blocks = [qkv_w[2 * C + (h0 + 2 * pr) * D: 2 * C + (h0 + 2 * pr) * D + 128]
                     for pr in range(PAIRS)]
        wv_host = np.ascontiguousarray(
            np.concatenate(wv_blocks, axis=0).T).astype(NP16)

        woT_host = np.ascontiguousarray(
            out_w[:, h0 * D:(h0 + HPC) * D].T).astype(NP16)
        bo_host = (np.tile(out_b, (128, 1)) if hg == 0
                   else np.zeros((128, C), np.float32)).astype(np.float32)

        in_maps.append({
            "xT": xt,
            "xT8": x8,
            "wqk8": np.ascontiguousarray(wqk),
            "wv": wv_host,
            "bqkv": bias_cols,
            "woT": woT_host,
            "bo": bo_host,
            "mask1": m1,
            "mask2": m2,
        })

    LAST_RESULT = run_bass_kernel_spmd(_NC, in_maps, core_ids=list(range(8)))
    res = LAST_RESULT.results
    out = np.empty((B, T, C), dtype=np.float32)
    for b in range(B):
        out[b] = res[b]["out"] + res[b + 4]["out"]
    return out
